# revision 1
# baseline (speedup 1.0000x reference)
"""Sparse graph-attention kernel for 8 TRN2 NeuronCores (Bass/Tile).

Problem (hardcoded): N=20000 nodes, E=640000 edges (src-sorted), Fin=256,
Fqk=256.  out[e] = exp(aw[e]) / segsum_src(exp(aw)),
aw[e] = (x[src[e]] @ Wq.T * Fqk**-0.5) . (x[dest[e]] @ Wk.T).

Key identity: aw[e] = p[src[e]] . x[dest[e]] with p = x @ G,
G = Fqk**-0.5 * Wq.T @ Wk (weight-only fold, computed host-side).
The k-side gathers raw x rows straight from the DRAM input (no k-table
projection or write); only the local p table (2560 rows) is computed on
device.

Sharding: src-node ranges (2500 nodes/core).  Per-node edge lists are
padded to multiples of 8 slots ("subrows") and packed into 128
partitions x 90 subrow-blocks, nodes never straddling a partition.
Gathers run in transpose mode (features land on partitions); the Pool
engine carries only the serial gather stream (its cost is bytes-rate in
the cost model) while loads/writes ride the sync/scalar engines
concurrently.  Per-edge products are one 2x DVE multiply per chunk and
the 256-dim dot reductions are PE ones-matmuls (contraction over
partitions) accumulating into PSUM aw columns.  Segment softmax with
two tensor_tensor_scans; mask-add and exp run per j-region, overlapped
with the gather stream.
"""

import numpy as np
import ml_dtypes

N = 20000
E = 640000
FIN = 256
FQK = 256
NCORES = 8
NLOC = N // NCORES          # 2500 nodes per core
SLOTS = 6                   # slots per subrow
BLK = 116                   # subrow blocks per partition
P = 128
RCAP = P * BLK              # 14848 subrows per core (capacity)
NSLOT = RCAP * SLOTS        # 89088 gather slots per core
QROWS = 2560                # 20*128, local p table rows
QT = QROWS // P             # 20 p-table node tiles
NQCH = 4                    # j-regions
GBLK = BLK // NQCH          # 29 subrow blocks per chunk/region
GCH = GBLK * P              # 3712 idxs per gather chunk
NKCH = SLOTS * NQCH         # 24 k-gather chunks
SEG = BLK * SLOTS           # 696 free positions per partition
RSEG = GBLK * SLOTS         # 174 positions per j-region

bf16 = ml_dtypes.bfloat16
_compiled = None            # cached compiled graph

# static 4-node-tile window base per 16-partition group for the q-side
# expansion matmuls (host asserts each core's packing fits the window)
TB = [min(max(int(round(2.4414 * g - 0.7793)), 0), QT - 4) for g in range(8)]


def _wrap_idx(vals):
    """int16 vals [n] (n % GCH == 0) -> wrapped dma_gather layout [128, n/16]:
    per chunk, idx j -> partition j%16 (replicated 8x), col j//16."""
    n = vals.shape[0]
    nch = n // GCH
    a = vals.reshape(nch, GCH // 16, 16).transpose(0, 2, 1)      # [nch,16,144]
    a = np.broadcast_to(a[:, None, :, :], (nch, 8, 16, GCH // 16))
    return np.ascontiguousarray(
        a.reshape(nch, 128, GCH // 16).transpose(1, 0, 2).reshape(128, n // 16)
    )


def _host_prep(x, ei, W):
    src = np.asarray(ei[0], np.int64)
    dest = np.asarray(ei[1], np.int64)
    x = np.asarray(x, np.float32)
    W = np.asarray(W, np.float32)

    # weight-only fold: aw[e] = (x[src] @ G) . x[dest]
    G = (FQK ** -0.5) * (W[:FQK].T @ W[FQK:])                    # [256, 256]
    Gb = np.ascontiguousarray(G.astype(bf16))

    xb = np.ascontiguousarray(x.astype(bf16))                    # gather table

    counts = np.bincount(src, minlength=N)
    starts = np.concatenate([[0], np.cumsum(counts)])            # [N+1]

    in_maps = []
    unshard = []
    for c in range(NCORES):
        n0 = c * NLOC
        # per-node subrow packing into partitions (none straddles a row)
        eg = np.full((P, BLK, SLOTS), -1, np.int64)              # edge ids
        seg_start = np.zeros((P, SEG), bool)
        seg_end = np.zeros((P, SEG), bool)
        p, b = 0, 0
        for n in range(n0, n0 + NLOC):
            d = int(counts[n])
            if d == 0:
                continue
            nsub = (d + SLOTS - 1) // SLOTS
            if b + nsub > BLK:
                if b < BLK:          # pad tail of this partition = 1 segment
                    seg_start[p, b * SLOTS] = True
                    seg_end[p, SEG - 1] = True
                p += 1
                b = 0
                assert p < P, "subrow capacity exceeded"
            e0 = starts[n]
            flat = eg[p, b:b + nsub].reshape(-1)
            flat[:d] = np.arange(e0, e0 + d)
            eg[p, b:b + nsub] = flat.reshape(nsub, SLOTS)
            seg_start[p, b * SLOTS] = True
            seg_end[p, (b + nsub) * SLOTS - 1] = True
            b += nsub
        if b < BLK:
            seg_start[p, b * SLOTS] = True
            seg_end[p, SEG - 1] = True
        for pp in range(p + 1, P):   # fully-pad partitions
            seg_start[pp, 0] = True
            seg_end[pp, SEG - 1] = True

        # k-side gather index values: absolute dest node ids, columns laid
        # out in chunk-issue order (j-major over regions, then slot s)
        egT = eg.transpose(2, 1, 0)                              # [8, 90, 128]
        valid = egT >= 0
        e_ids = np.where(valid, egT, 0)
        kval = np.where(valid, dest[e_ids], 0).astype(np.int16)
        kseq = kval.reshape(SLOTS, NQCH, GBLK, P).transpose(1, 0, 2, 3)
        kg_idx = _wrap_idx(np.ascontiguousarray(kseq).reshape(-1))  # [128,5760]

        # q-side: local src id per subrow (order: block-major, partition)
        # q-side expansion matrices: per (region j, 16-partition group g),
        # one-hot E so that  qt[:, fh, cols] = p_tiles @ E  (PE matmuls).
        # E slice covers cols (p_local, b_local) = p_local*18 + b_local and
        # node rows [128*TB[g], 128*TB[g] + 512) (4 node tiles, static).
        eg0 = eg[:, :, 0]                                        # [128, 90]
        v2 = eg0 >= 0
        qv = np.where(v2, src[np.where(v2, eg0, 0)] - n0, 0).astype(np.int32)
        NG = P // 16                                             # 8 groups
        Etab = np.zeros((NQCH, NG, 4 * P, GBLK * 16), bf16)
        for g in range(NG):
            pv = qv[16 * g:16 * (g + 1)]                         # [16, 90]
            vv = v2[16 * g:16 * (g + 1)]
            if vv.any():
                lo, hi = pv[vv].min(), pv[vv].max()
                assert 128 * TB[g] <= lo and hi < 128 * TB[g] + 512, \
                    (g, lo, hi, TB[g])
            for j in range(NQCH):
                for pl in range(16):
                    for bl in range(GBLK):
                        if vv[pl, GBLK * j + bl]:
                            r = pv[pl, GBLK * j + bl] - 128 * TB[g]
                            Etab[j, g, r, pl * GBLK + bl] = 1.0
        Etab = np.ascontiguousarray(Etab.reshape(NQCH * NG * 4 * P, GBLK * 16))

        maskB = np.where(eg.reshape(P, SEG) >= 0, 0.0, -30.0).astype(np.float32)
        cmask = np.where(seg_start, 0.0, 1.0).astype(np.float32)
        emask_rev = np.ascontiguousarray(
            np.where(seg_end, 0.0, 1.0).astype(np.float32)[:, ::-1])

        # local x slice, transposed for the p matmul
        xl = np.zeros((QROWS, FIN), np.float32)
        hi = min(n0 + QROWS, N)
        xl[:hi - n0] = x[n0:hi]
        xlT = np.ascontiguousarray(xl.T.astype(bf16))            # [256, 2560]

        in_maps.append(dict(xb=xb, xlT=xlT, G=Gb, kg_idx=kg_idx, Etab=Etab,
                            maskB=maskB, cmask=cmask, emask_rev=emask_rev))

        pm, bm, sm = np.where(eg >= 0)
        unshard.append((eg[pm, bm, sm], pm, bm * SLOTS + sm))
    return in_maps, unshard


def _build():
    import concourse.bacc as bacc
    import concourse.mybir as mybir
    import concourse.tile as tile
    from concourse import library_config
    from concourse.tile_rust import add_dep_helper

    fp32 = mybir.dt.float32
    b16 = mybir.dt.bfloat16
    Alu = mybir.AluOpType

    nc = bacc.Bacc("TRN2", target_bir_lowering=False, debug=False,
                   dynamic_dma_scratch_size=16384)
    xb_d = nc.dram_tensor("xb", [N, FIN], b16, kind="ExternalInput")
    xlT_d = nc.dram_tensor("xlT", [FIN, QROWS], b16, kind="ExternalInput")
    G_d = nc.dram_tensor("G", [FIN, FQK], b16, kind="ExternalInput")
    kgi_d = nc.dram_tensor("kg_idx", [P, NSLOT // 16], mybir.dt.int16,
                           kind="ExternalInput")
    Etab_d = nc.dram_tensor("Etab", [NQCH * 8 * 4 * P, GBLK * 16], b16,
                            kind="ExternalInput")
    mB_d = nc.dram_tensor("maskB", [P, SEG], fp32, kind="ExternalInput")
    cm_d = nc.dram_tensor("cmask", [P, SEG], fp32, kind="ExternalInput")
    em_d = nc.dram_tensor("emask_rev", [P, SEG], fp32, kind="ExternalInput")
    out_d = nc.dram_tensor("out", [P, SEG], fp32, kind="ExternalOutput")

    ICOL = GCH // 16                                 # idx cols per chunk (144)

    with tile.TileContext(nc) as tc:
        with tc.tile_pool(name="persist", bufs=1) as sb, \
             tc.tile_pool(name="kg", bufs=5) as kgsb, \
             tc.tile_pool(name="qtp", bufs=3) as qtp, \
             tc.tile_pool(name="xlt", bufs=1) as xltp, \
             tc.tile_pool(name="ep", bufs=3) as epool, \
             tc.tile_pool(name="mps", bufs=1, space="PSUM") as mps, \
             tc.tile_pool(name="eps", bufs=2, space="PSUM") as epsp, \
             tc.tile_pool(name="awps", bufs=1, space="PSUM") as awps:
            lib = nc.gpsimd.load_library(library_config.mlp)

            # --- input loads: kgi on sync (feeds Pool stream), rest scalar ---
            kgi = sb.tile([P, NSLOT // 16], mybir.dt.int16)
            nc.sync.dma_start(kgi[:, :ICOL], kgi_d[:, :ICOL])
            nc.sync.dma_start(kgi[:, ICOL:], kgi_d[:, ICOL:])
            Gt = sb.tile([P, 2, FQK], b16)
            nc.scalar.dma_start(Gt[:], G_d[:, :].rearrange("(c p) f -> p c f", p=P))
            mB = sb.tile([P, SEG], fp32)
            cm = sb.tile([P, SEG], fp32)
            em = sb.tile([P, SEG], fp32)
            ones = sb.tile([P, 1], b16)
            nc.vector.memset(ones[:], 1.0)

            xlT = xltp.tile([P, 2, QROWS], b16)
            nc.scalar.dma_start(
                xlT[:], xlT_d[:, :].rearrange("(c p) f -> p c f", p=P))
            nc.sync.dma_start(mB[:], mB_d[:])
            nc.sync.dma_start(cm[:], cm_d[:])
            nc.sync.dma_start(em[:], em_d[:])

            # --- p projection: p = x_loc @ G, 20 node tiles, groups of 2;
            # results stay in SBUF (pt tiles) for the expansion matmuls ---
            pts = [sb.tile([P, 2, FQK], b16, name=f"pt{g}")
                   for g in range(QT // 2)]
            for g0 in range(0, QT, 2):
                acc = mps.tile([P, 2 * FQK], fp32)
                for t in range(2):
                    nt = g0 + t
                    sl = slice(nt * P, (nt + 1) * P)
                    ao = t * FQK
                    nc.tensor.matmul(acc[:, ao:ao + FQK],
                                     lhsT=xlT[:, 0, sl],
                                     rhs=Gt[:, 0, :],
                                     start=True, stop=False)
                    nc.tensor.matmul(acc[:, ao:ao + FQK],
                                     lhsT=xlT[:, 1, sl],
                                     rhs=Gt[:, 1, :],
                                     start=False, stop=True)
                if (g0 // 2) % 2 == 0:
                    nc.scalar.copy(pts[g0 // 2][:], acc[:])
                else:
                    nc.vector.tensor_copy(pts[g0 // 2][:], acc[:])

            def pt_half(nt, fh):
                # lhsT [128 nodes, 128 f] for node tile nt, feature half fh
                return pts[nt // 2][:, nt % 2, fh * P:(fh + 1) * P]

            # --- aw accumulators in PSUM: one [P, 144] tile per j-region ---
            aw_ps = [awps.tile([P, RSEG], fp32, name=f"aw_ps{j}")
                     for j in range(NQCH)]

            ex = sb.tile([P, SEG], fp32)

            def k_gather(s, j, b0=0, b1=GBLK):
                n = (b1 - b0) * P
                kt = kgsb.tile([P, 2, n], b16, tag="kt")
                co = (j * SLOTS + s) * ICOL + b0 * 8
                g = nc.gpsimd.dma_gather(
                    kt[:], xb_d[:, :], kgi[:, co:co + n // 16],
                    n, n, FQK, single_packet=False, transpose=True)
                add_dep_helper(lib.ins, g.ins, sync=True, reason="lib first")
                return kt

            def k_compute(kt, qt, s, j, b0=0, b1=GBLK):
                # per-edge products (in place, 2x DVE)
                nc.vector.tensor_tensor(
                    out=kt[:], in0=kt[:],
                    in1=qt[:, :, b0 * P:b1 * P], op=Alu.mult)
                # 256-dim dot per block: PE ones-matmul partition reduce
                for b in range(b0, b1):
                    pos = b * SLOTS + s
                    for h in (0, 1):
                        nc.tensor.matmul(
                            aw_ps[j][:, pos:pos + 1],
                            lhsT=kt[:, h, (b - b0) * P:(b - b0 + 1) * P],
                            rhs=ones[:, 0:1],
                            start=(h == 0), stop=(h == 1))

            ECOL = GBLK * 16                         # 288 cols per E slice

            def e_region(j):
                # produce qt[j] = p-rows-per-subrow via one-hot expansion
                # matmuls (PE) from the SBUF p tiles; copies remap cols to
                # (b*128 + p) order while casting PSUM fp32 -> bf16
                qt = qtp.tile([P, 2, GCH], b16, tag="qt")
                for g in range(8):
                    Et = epool.tile([P, 4, ECOL], b16, tag="Et")
                    ro = (j * 8 + g) * 4 * P
                    eng = nc.sync if g % 2 == 0 else nc.scalar
                    eng.dma_start(Et[:], Etab_d[ro:ro + 4 * P, :].rearrange(
                        "(k p) c -> p k c", p=P))
                    for fh in (0, 1):
                        eps = epsp.tile([P, ECOL], fp32, tag="eps")
                        for k in range(4):
                            nc.tensor.matmul(eps[:],
                                             lhsT=pt_half(TB[g] + k, fh),
                                             rhs=Et[:, k, :],
                                             start=(k == 0), stop=(k == 3))
                        dst = qt[:, fh, :].rearrange(
                            "p (b q) -> p q b", b=GBLK)[:, 16 * g:16 * (g + 1), :]
                        srcv = eps[:].rearrange("p (q b) -> p q b", q=16)
                        nc.scalar.copy(dst, srcv)
                return qt

            # --- main stream: Pool runs pure k-gathers; qt regions are
            # produced by PE/copies just ahead of their consumption ---
            for j in range(NQCH):
                qt = e_region(j)
                awm_t = sb.tile([P, RSEG], fp32, name=f"awm{j}")

                def r_finish(b0, b1, j=j, awm_t=awm_t):
                    # mask-add + exp for region cols [b0*SLOTS, b1*SLOTS)
                    c0, c1 = b0 * SLOTS, b1 * SLOTS
                    rs = slice(j * RSEG + c0, j * RSEG + c1)
                    nc.vector.tensor_tensor(out=awm_t[:, c0:c1],
                                            in0=aw_ps[j][:, c0:c1],
                                            in1=mB[:, rs], op=Alu.add)
                    nc.scalar.activation(ex[:, rs], awm_t[:, c0:c1],
                                         mybir.ActivationFunctionType.Exp)

                for s in range(SLOTS):
                    if j == NQCH - 1 and s == SLOTS - 1:
                        # split the final chunk so its dots overlap the tail
                        for b0, b1 in ((0, 10), (10, 20), (20, GBLK)):
                            kt = k_gather(s, j, b0, b1)
                            k_compute(kt, qt, s, j, b0, b1)
                        continue
                    kt = k_gather(s, j)
                    k_compute(kt, qt, s, j)
                # region j complete: mask-add + exp now (overlapped)
                r_finish(0, GBLK)

            # --- segment softmax (scans are global over SEG) ---
            with tc.tile_pool(name="smx", bufs=1) as smx:
                pfx = smx.tile([P, SEG], fp32)
                nc.vector.tensor_tensor_scan(
                    out=pfx[:], data0=cm[:], data1=ex[:], initial=0.0,
                    op0=Alu.mult, op1=Alu.add)
                tot = smx.tile([P, SEG], fp32)
                nc.vector.tensor_tensor_scan(
                    out=tot[:, ::-1], data0=em[:], data1=pfx[:, ::-1], initial=0.0,
                    op0=Alu.mult, op1=Alu.max)
                rec = smx.tile([P, SEG], fp32)
                nc.vector.reciprocal_approx_fast(rec[:], tot[:])
                o = smx.tile([P, SEG], fp32)
                h = SEG // 2
                nc.vector.tensor_tensor(out=o[:, :h], in0=ex[:, :h],
                                        in1=rec[:, :h], op=Alu.mult)
                nc.sync.dma_start(out_d[:, :h], o[:, :h])
                nc.vector.tensor_tensor(out=o[:, h:], in0=ex[:, h:],
                                        in1=rec[:, h:], op=Alu.mult)
                nc.scalar.dma_start(out_d[:, h:], o[:, h:])
    nc.compile()
    return nc


def kernel(x, ei, W):
    global _compiled
    in_maps, unshard = _host_prep(x, ei, W)
    if _compiled is None:
        _compiled = _build()
    nc = _compiled
    from concourse.bass_utils import run_bass_kernel_spmd
    res = run_bass_kernel_spmd(nc, in_maps, core_ids=list(range(NCORES)))
    out = np.empty(E, np.float32)
    for c in range(NCORES):
        eids, pm, tm = unshard[c]
        out[eids] = res.results[c]["out"][pm, tm]
    return out



# revision 12
# speedup vs baseline: 1.5653x; 1.5653x over previous
"""Sparse graph-attention kernel for 8 TRN2 NeuronCores (Bass/Tile).

Problem (hardcoded): N=20000 nodes, E=640000 edges (src-sorted), Fin=256,
Fqk=256.  out[e] = exp(aw[e]) / segsum_src(exp(aw)),
aw[e] = (x[src[e]] @ Wq.T * Fqk**-0.5) . (x[dest[e]] @ Wk.T).

Key identity: aw[e] = p[src[e]] . x[dest[e]] with p = x @ G,
G = Fqk**-0.5 * Wq.T @ Wk (weight-only fold, computed host-side).

Transport: the full x table is held in SBUF as an int32-paired feature-major
table ktT[p, n] = (bf16 x[n, 2p], bf16 x[n, 2p+1]), so one gpsimd ap_gather
element per edge moves the whole 512B k-row (features across partitions).
Edges are packed per src node into capacity-class columns (capacity =
ceil(deg/4)*4, schedule = pointwise max of per-core sorted class lists, so
one compiled graph serves all cores); per node, two PE matmuls (even/odd
feature planes, stride-2 lhsT over the gathered pairs) against the node's
projected p-column produce the per-edge dots directly in PSUM [deg, col] --
no per-slot q expansion and no elementwise multiply pass.  Softmax per
column: mask-add, exp (Act), partition-sum via ones-matmul, reciprocal,
K=1-matmul broadcast, and one elementwise divide.

Sharding: src-node ranges (2500 nodes/core); each core gathers its dest
rows from the replicated SBUF table.
"""

import numpy as np
import ml_dtypes

N = 20000
E = 640000
FIN = 256
FQK = 256
NCORES = 8
NLOC = N // NCORES          # 2500 nodes per core
CL = 4                      # capacity class granularity
NCH = 4                     # gather chunks (each >= N idxs for full rate)
QROWS = 2560                # pT/xlT column capacity (>= NLOC)
P = 128
REG = 512                   # softmax region width (PSUM bank cols)

bf16 = ml_dtypes.bfloat16
_compiled = None
_sched = None               # (slotcls, chunk col ranges, chunk slot counts, offsets)


def _wrap_idx(vals):
    """int16 vals [n] (n % 16 == 0) -> ap_gather idx layout [128, n/16]:
    idx j -> partition j%16 (replicated across the 8 groups), col j//16."""
    n = vals.shape[0]
    a = vals.reshape(n // 16, 16).T                      # [16, n/16]
    return np.ascontiguousarray(np.tile(a, (8, 1)).astype(np.int16))


def _schedule(all_counts):
    """Static schedule from per-core degree lists (identical for all cores).

    Returns (slotcls [NLOC], chunks [(c0, c1, nslots)], sloto [NLOC]) where
    sloto[i] is column i's slot offset inside its chunk."""
    slotcls = np.zeros(NLOC, np.int64)
    for c in range(NCORES):
        d = all_counts[c * NLOC:(c + 1) * NLOC]
        cls = np.sort(-(-(d) // CL) * CL)[::-1]          # descending classes
        assert cls.max() <= P, "node degree exceeds one PSUM column"
        slotcls = np.maximum(slotcls, cls)
    cum = np.cumsum(slotcls)
    bnds = [0]
    for k in range(1, NCH):
        bnds.append(int(np.searchsorted(cum, cum[-1] * k / NCH)))
    bnds.append(NLOC)
    chunks = []
    sloto = np.zeros(NLOC, np.int64)
    for k in range(NCH):
        c0, c1 = bnds[k], bnds[k + 1]
        off = 0
        for i in range(c0, c1):
            sloto[i] = off
            off += int(slotcls[i])
        nsl = -(-off // 16) * 16                         # pad to 16
        assert nsl >= N, "chunk below table-size floor; retune NCH"
        chunks.append((c0, c1, nsl))
    return slotcls, chunks, sloto


def _host_prep(x, ei, W):
    global _sched
    src = np.asarray(ei[0], np.int64)
    dest = np.asarray(ei[1], np.int64)
    x = np.asarray(x, np.float32)
    W = np.asarray(W, np.float32)

    # weight-only fold: aw[e] = (x[src] @ G) . x[dest]; even/odd G columns
    G = (FQK ** -0.5) * (W[:FQK].T @ W[FQK:])            # [256, 256]
    Ge = np.ascontiguousarray(G[:, 0::2].astype(bf16))   # [256, 128]
    Go = np.ascontiguousarray(G[:, 1::2].astype(bf16))

    # feature-major int32-paired gather table (same for all cores)
    xb = x.astype(bf16)                                  # [N, 256]
    ktT = np.ascontiguousarray(
        xb.reshape(N, P, 2).transpose(1, 0, 2)).view(np.int32).reshape(P, N)

    counts = np.bincount(src, minlength=N)
    starts = np.concatenate([[0], np.cumsum(counts)])    # [N+1]

    slotcls, chunks, sloto = _schedule(counts)
    _sched = (slotcls, chunks, sloto)
    totslots = sum(nsl for _, _, nsl in chunks)

    in_maps = []
    unshard = []
    for c in range(NCORES):
        n0 = c * NLOC
        d = counts[n0:n0 + NLOC]
        cls = -(-d // CL) * CL
        order = np.argsort(-cls, kind="stable")          # rank i -> local node
        assert (cls[order] <= slotcls).all(), "schedule infeasible"

        wraps = []
        for (c0, c1, nsl) in chunks:
            seg = np.zeros(nsl, np.int16)
            for i in range(c0, c1):
                n = n0 + order[i]
                dg = int(counts[n])
                o = int(sloto[i])
                seg[o:o + dg] = dest[starts[n]:starts[n] + dg].astype(np.int16)
            wraps.append(_wrap_idx(seg))
        kgi_w = np.ascontiguousarray(np.concatenate(wraps, axis=1))

        xl = np.zeros((QROWS, FIN), np.float32)
        xl[:NLOC] = x[n0 + order]
        xlT = np.ascontiguousarray(xl.T.astype(bf16))    # [256, QROWS]

        mB = np.full((P, QROWS), -30.0, np.float32)
        for i in range(NLOC):
            mB[:d[order[i]], i] = 0.0

        in_maps.append(dict(ktT=ktT, kgi=kgi_w, xlT=xlT, Ge=Ge, Go=Go, mB=mB))
        unshard.append((order, starts[n0:n0 + NLOC + 1].copy(), d))
    return in_maps, unshard


def _build():
    import concourse.bacc as bacc
    import concourse.mybir as mybir
    import concourse.tile as tile
    from concourse import library_config
    from concourse.tile_rust import add_dep_helper

    fp32 = mybir.dt.float32
    b16 = mybir.dt.bfloat16
    i32 = mybir.dt.int32
    i16 = mybir.dt.int16
    Alu = mybir.AluOpType

    slotcls, chunks, sloto = _sched
    totslots = sum(nsl for _, _, nsl in chunks)
    maxch = max(nsl for _, _, nsl in chunks)

    nc = bacc.Bacc("TRN2", target_bir_lowering=False, debug=False)
    ktT_d = nc.dram_tensor("ktT", [P, N], i32, kind="ExternalInput")
    kgi_d = nc.dram_tensor("kgi", [P, totslots // 16], i16, kind="ExternalInput")
    xlT_d = nc.dram_tensor("xlT", [FIN, QROWS], b16, kind="ExternalInput")
    Ge_d = nc.dram_tensor("Ge", [FIN, P], b16, kind="ExternalInput")
    Go_d = nc.dram_tensor("Go", [FIN, P], b16, kind="ExternalInput")
    mB_d = nc.dram_tensor("mB", [P, QROWS], fp32, kind="ExternalInput")
    out_d = nc.dram_tensor("out", [P, QROWS], fp32, kind="ExternalOutput")

    with tile.TileContext(nc) as tc:
        with tc.tile_pool(name="persist", bufs=1) as sb, \
             tc.tile_pool(name="ktp", bufs=1) as ktp, \
             tc.tile_pool(name="kgip", bufs=2) as kgip, \
             tc.tile_pool(name="reg", bufs=2) as rp, \
             tc.tile_pool(name="recp", bufs=1) as recp, \
             tc.tile_pool(name="aws", bufs=2, space="PSUM") as awps, \
             tc.tile_pool(name="esps", bufs=2, space="PSUM") as esps, \
             tc.tile_pool(name="rbps", bufs=2, space="PSUM") as rbps:
            lib = nc.gpsimd.load_library(library_config.ap_gather)

            # --- persistent inputs; table load split SP/Act/Pool (the only
            # DMA-capable engines); non-critical loads follow the table ---
            ktT = sb.tile([P, N, 1], i32)
            b0, b1 = 6500, 13500
            nc.sync.dma_start(ktT[:, :b0, :],
                              ktT_d[:, :b0].rearrange("p (n d) -> p n d", d=1))
            nc.scalar.dma_start(ktT[:, b0:b1, :],
                                ktT_d[:, b0:b1].rearrange("p (n d) -> p n d", d=1))
            nc.gpsimd.dma_start(ktT[:, b1:, :],
                                ktT_d[:, b1:].rearrange("p (n d) -> p n d", d=1))
            xlT = sb.tile([P, 2, QROWS], b16)
            nc.scalar.dma_start(xlT[:], xlT_d[:, :].rearrange("(c p) f -> p c f", p=P))
            Ge = sb.tile([P, 2, P], b16)
            Go = sb.tile([P, 2, P], b16)
            nc.scalar.dma_start(Ge[:], Ge_d[:, :].rearrange("(c p) f -> p c f", p=P))
            nc.scalar.dma_start(Go[:], Go_d[:, :].rearrange("(c p) f -> p c f", p=P))
            mB = sb.tile([P, QROWS], fp32)
            nc.sync.dma_start(mB[:], mB_d[:])
            onesf = sb.tile([P, 1], fp32)
            nc.vector.memset(onesf[:], 1.0)
            ones1 = sb.tile([1, P], fp32)
            nc.vector.memset(ones1[:], 1.0)
            maxch16 = max(nsl for _, _, nsl in chunks) // 16

            # --- p projection in even/odd planes: pT_u[p, i] = q[i, 2p+u] ---
            pTe = sb.tile([P, QROWS], b16)
            pTo = sb.tile([P, QROWS], b16)
            for u, (Gt, pT) in enumerate(((Ge, pTe), (Go, pTo))):
                for o in range(0, QROWS, REG):
                    acc = awps.tile([P, REG], fp32, tag="proj")
                    nc.tensor.matmul(acc[:], lhsT=Gt[:, 0, :],
                                     rhs=xlT[:, 0, o:o + REG], start=True, stop=False)
                    nc.tensor.matmul(acc[:], lhsT=Gt[:, 1, :],
                                     rhs=xlT[:, 1, o:o + REG], start=False, stop=True)
                    nc.scalar.copy(pT[:, o:o + REG], acc[:])

            # --- main stream: per chunk, one ap_gather then per-node dots ---
            soff16 = 0
            for k, (c0, c1, nsl) in enumerate(chunks):
                kgi_t = kgip.tile([P, maxch16], i16, tag="kgi")
                nc.sync.dma_start(kgi_t[:, :nsl // 16],
                                  kgi_d[:, soff16 // 16:(soff16 + nsl) // 16])
                kt = ktp.tile([P, maxch, 1], i32, tag="kt")
                g = nc.gpsimd.ap_gather(
                    kt[:, :nsl, :], ktT[:], kgi_t[:, :nsl // 16],
                    P, N, 1, nsl)
                add_dep_helper(lib.ins, g.ins, sync=True, reason="lib first")
                soff16 += nsl
                ktv = kt[:].bitcast(b16)                 # [P, maxch, 2]

                ncols = c1 - c0
                nreg = -(-ncols // REG)
                rw = -(-ncols // nreg)                   # region width
                for r0 in range(c0, c1, rw):
                    r1 = min(r0 + rw, c1)
                    w = r1 - r0
                    aw = awps.tile([P, REG], fp32, tag="aw")
                    nc.vector.memset(aw[:], -30.0)
                    for i in range(r0, r1):
                        ccap = int(slotcls[i])
                        o = int(sloto[i])
                        j = i - r0
                        nc.tensor.matmul(aw[0:ccap, j:j + 1],
                                         lhsT=ktv[:, o:o + ccap, 0],
                                         rhs=pTe[:, i:i + 1],
                                         start=True, stop=False)
                        nc.tensor.matmul(aw[0:ccap, j:j + 1],
                                         lhsT=ktv[:, o:o + ccap, 1],
                                         rhs=pTo[:, i:i + 1],
                                         start=False, stop=True)
                    awm = rp.tile([P, REG], fp32, tag="awm")
                    nc.vector.tensor_tensor(out=awm[:, :w], in0=aw[:, :w],
                                            in1=mB[:, r0:r1], op=Alu.add)
                    ex = rp.tile([P, REG], fp32, tag="ex")
                    nc.scalar.activation(ex[:, :w], awm[:, :w],
                                         mybir.ActivationFunctionType.Exp)
                    es = esps.tile([1, REG], fp32, tag="es")
                    nc.tensor.matmul(es[0:1, :w], lhsT=onesf[:, 0:1],
                                     rhs=ex[:, :w], start=True, stop=True)
                    rec = recp.tile([1, REG], fp32, tag="rec")
                    nc.vector.reciprocal_approx_fast(rec[0:1, :w], es[0:1, :w])
                    rb = rbps.tile([P, REG], fp32, tag="rb")
                    nc.tensor.matmul(rb[:, :w], lhsT=ones1[:],
                                     rhs=rec[0:1, :w], start=True, stop=True)
                    o_t = rp.tile([P, REG], fp32, tag="awm")
                    nc.vector.tensor_tensor(out=o_t[:, :w], in0=ex[:, :w],
                                            in1=rb[:, :w], op=Alu.mult)
                    eng = nc.sync if (r0 // rw) % 2 == 0 else nc.scalar
                    eng.dma_start(out_d[:, r0:r1], o_t[:, :w])
    nc.compile()
    return nc


def kernel(x, ei, W):
    global _compiled
    in_maps, unshard = _host_prep(x, ei, W)
    if _compiled is None:
        _compiled = _build()
    nc = _compiled
    from concourse.bass_utils import run_bass_kernel_spmd
    res = run_bass_kernel_spmd(nc, in_maps, core_ids=list(range(NCORES)))
    out = np.empty(E, np.float32)
    for c in range(NCORES):
        order, starts_l, d = unshard[c]
        o = res.results[c]["out"]
        for i in range(NLOC):
            n = order[i]
            dg = int(d[n])
            out[starts_l[n]:starts_l[n] + dg] = o[:dg, i]
    return out


# revision 13
# speedup vs baseline: 1.6305x; 1.0416x over previous
"""Sparse graph-attention kernel for 8 TRN2 NeuronCores (Bass/Tile).

Problem (hardcoded): N=20000 nodes, E=640000 edges (src-sorted), Fin=256,
Fqk=256.  out[e] = exp(aw[e]) / segsum_src(exp(aw)),
aw[e] = (x[src[e]] @ Wq.T * Fqk**-0.5) . (x[dest[e]] @ Wk.T).

Key identity: aw[e] = p[src[e]] . x[dest[e]] with p = x @ G,
G = Fqk**-0.5 * Wq.T @ Wk (weight-only fold, computed host-side).

Transport: the full x table is held in SBUF as an int32-paired feature-major
table ktT[p, n] = (bf16 x[n, 2p], bf16 x[n, 2p+1]), so one gpsimd ap_gather
element per edge moves the whole 512B k-row (features across partitions).
Edges are packed per src node into capacity-class columns (capacity =
ceil(deg/4)*4, schedule = pointwise max of per-core sorted class lists, so
one compiled graph serves all cores); per node, two PE matmuls (even/odd
feature planes, stride-2 lhsT over the gathered pairs) against the node's
projected p-column produce the per-edge dots directly in PSUM [deg, col] --
no per-slot q expansion and no elementwise multiply pass.  Softmax per
column: mask-add, exp (Act), partition-sum via ones-matmul, reciprocal,
K=1-matmul broadcast, and one elementwise divide.

Sharding: src-node ranges (2500 nodes/core); each core gathers its dest
rows from the replicated SBUF table.
"""

import numpy as np
import ml_dtypes

N = 20000
E = 640000
FIN = 256
FQK = 256
NCORES = 8
NLOC = N // NCORES          # 2500 nodes per core
CL = 1                      # capacity class granularity (exact degrees)
NCH = 4                     # gather chunks (each >= N idxs for full rate)
QROWS = 2560                # pT/xlT column capacity (>= NLOC)
P = 128
REG = 256                   # softmax region width (PSUM cols)

bf16 = ml_dtypes.bfloat16
_compiled = None
_sched = None               # (slotcls, chunk col ranges, chunk slot counts, offsets)


def _wrap_idx(vals):
    """int16 vals [n] (n % 16 == 0) -> ap_gather idx layout [128, n/16]:
    idx j -> partition j%16 (replicated across the 8 groups), col j//16."""
    n = vals.shape[0]
    a = vals.reshape(n // 16, 16).T                      # [16, n/16]
    return np.ascontiguousarray(np.tile(a, (8, 1)).astype(np.int16))


def _schedule(all_counts):
    """Static schedule from per-core degree lists (identical for all cores).

    Returns (slotcls [NLOC], chunks [(c0, c1, nslots)], sloto [NLOC]) where
    sloto[i] is column i's slot offset inside its chunk."""
    slotcls = np.zeros(NLOC, np.int64)
    for c in range(NCORES):
        d = all_counts[c * NLOC:(c + 1) * NLOC]
        cls = np.sort(-(-(d) // CL) * CL)[::-1]          # descending classes
        assert cls.max() <= P, "node degree exceeds one PSUM column"
        slotcls = np.maximum(slotcls, cls)
    slotcls = slotcls[::-1].copy()                       # ascending
    cum = np.cumsum(slotcls)
    bnds = [0]
    for k in range(1, NCH):
        bnds.append(int(np.searchsorted(cum, cum[-1] * k / NCH)))
    bnds.append(NLOC)
    chunks = []
    sloto = np.zeros(NLOC, np.int64)
    for k in range(NCH):
        c0, c1 = bnds[k], bnds[k + 1]
        off = 0
        for i in range(c0, c1):
            sloto[i] = off
            off += int(slotcls[i])
        nsl = -(-off // 16) * 16                         # pad to 16
        assert nsl >= N, "chunk below table-size floor; retune NCH"
        chunks.append((c0, c1, nsl))
    return slotcls, chunks, sloto


def _host_prep(x, ei, W):
    global _sched
    src = np.asarray(ei[0], np.int64)
    dest = np.asarray(ei[1], np.int64)
    x = np.asarray(x, np.float32)
    W = np.asarray(W, np.float32)

    # weight-only fold: aw[e] = (x[src] @ G) . x[dest]; even/odd G columns
    G = (FQK ** -0.5) * (W[:FQK].T @ W[FQK:])            # [256, 256]
    Ge = np.ascontiguousarray(G[:, 0::2].astype(bf16))   # [256, 128]
    Go = np.ascontiguousarray(G[:, 1::2].astype(bf16))

    # feature-major int32-paired gather table (same for all cores)
    xb = x.astype(bf16)                                  # [N, 256]
    ktT = np.ascontiguousarray(
        xb.reshape(N, P, 2).transpose(1, 0, 2)).view(np.int32).reshape(P, N)

    counts = np.bincount(src, minlength=N)
    starts = np.concatenate([[0], np.cumsum(counts)])    # [N+1]

    slotcls, chunks, sloto = _schedule(counts)
    _sched = (slotcls, chunks, sloto)
    totslots = sum(nsl for _, _, nsl in chunks)

    in_maps = []
    unshard = []
    for c in range(NCORES):
        n0 = c * NLOC
        d = counts[n0:n0 + NLOC]
        cls = -(-d // CL) * CL
        order = np.argsort(-cls, kind="stable")[::-1]    # rank i -> local node
        assert (cls[order] <= slotcls).all(), "schedule infeasible"

        wraps = []
        for (c0, c1, nsl) in chunks:
            seg = np.zeros(nsl, np.int16)
            for i in range(c0, c1):
                n = n0 + order[i]
                dg = int(counts[n])
                o = int(sloto[i])
                seg[o:o + dg] = dest[starts[n]:starts[n] + dg].astype(np.int16)
            wraps.append(_wrap_idx(seg))
        kgi_w = np.ascontiguousarray(np.concatenate(wraps, axis=1))

        xl = np.zeros((QROWS, FIN), np.float32)
        xl[:NLOC] = x[n0 + order]
        xlT = np.ascontiguousarray(xl.T.astype(bf16))    # [256, QROWS]

        mB = np.full((P, QROWS), -30.0, np.float32)
        for i in range(NLOC):
            mB[:d[order[i]], i] = 0.0

        in_maps.append(dict(ktT=ktT, kgi=kgi_w, xlT=xlT, Ge=Ge, Go=Go, mB=mB))
        unshard.append((order, starts[n0:n0 + NLOC + 1].copy(), d))
    return in_maps, unshard


def _build():
    import concourse.bacc as bacc
    import concourse.mybir as mybir
    import concourse.tile as tile
    from concourse import library_config
    from concourse.tile_rust import add_dep_helper

    fp32 = mybir.dt.float32
    b16 = mybir.dt.bfloat16
    i32 = mybir.dt.int32
    i16 = mybir.dt.int16
    Alu = mybir.AluOpType

    slotcls, chunks, sloto = _sched
    totslots = sum(nsl for _, _, nsl in chunks)
    maxch = max(nsl for _, _, nsl in chunks)

    nc = bacc.Bacc("TRN2", target_bir_lowering=False, debug=False)
    ktT_d = nc.dram_tensor("ktT", [P, N], i32, kind="ExternalInput")
    kgi_d = nc.dram_tensor("kgi", [P, totslots // 16], i16, kind="ExternalInput")
    xlT_d = nc.dram_tensor("xlT", [FIN, QROWS], b16, kind="ExternalInput")
    Ge_d = nc.dram_tensor("Ge", [FIN, P], b16, kind="ExternalInput")
    Go_d = nc.dram_tensor("Go", [FIN, P], b16, kind="ExternalInput")
    mB_d = nc.dram_tensor("mB", [P, QROWS], fp32, kind="ExternalInput")
    out_d = nc.dram_tensor("out", [P, QROWS], fp32, kind="ExternalOutput")

    with tile.TileContext(nc) as tc:
        with tc.tile_pool(name="persist", bufs=1) as sb, \
             tc.tile_pool(name="ktp", bufs=1) as ktp, \
             tc.tile_pool(name="kgip", bufs=2) as kgip, \
             tc.tile_pool(name="reg", bufs=2) as rp, \
             tc.tile_pool(name="recp", bufs=1) as recp, \
             tc.tile_pool(name="aws", bufs=2, space="PSUM") as awps, \
             tc.tile_pool(name="esps", bufs=2, space="PSUM") as esps, \
             tc.tile_pool(name="rbps", bufs=2, space="PSUM") as rbps:
            lib = nc.gpsimd.load_library(library_config.ap_gather)

            # --- persistent inputs; table load split SP/Act/Pool (the only
            # DMA-capable engines); non-critical loads follow the table ---
            ktT = sb.tile([P, N, 1], i32)
            b0, b1 = 7100, 13800
            nc.sync.dma_start(ktT[:, :b0, :],
                              ktT_d[:, :b0].rearrange("p (n d) -> p n d", d=1))
            nc.scalar.dma_start(ktT[:, b0:b1, :],
                                ktT_d[:, b0:b1].rearrange("p (n d) -> p n d", d=1))
            nc.gpsimd.dma_start(ktT[:, b1:, :],
                                ktT_d[:, b1:].rearrange("p (n d) -> p n d", d=1))
            xlT = sb.tile([P, 2, QROWS], b16)
            nc.scalar.dma_start(xlT[:], xlT_d[:, :].rearrange("(c p) f -> p c f", p=P))
            Ge = sb.tile([P, 2, P], b16)
            Go = sb.tile([P, 2, P], b16)
            nc.scalar.dma_start(Ge[:], Ge_d[:, :].rearrange("(c p) f -> p c f", p=P))
            nc.scalar.dma_start(Go[:], Go_d[:, :].rearrange("(c p) f -> p c f", p=P))
            mB = sb.tile([P, QROWS], fp32)
            nc.scalar.dma_start(mB[:], mB_d[:])
            onesf = sb.tile([P, 1], fp32)
            nc.vector.memset(onesf[:], 1.0)
            ones1 = sb.tile([1, P], fp32)
            nc.vector.memset(ones1[:], 1.0)
            maxch16 = max(nsl for _, _, nsl in chunks) // 16

            # --- p projection in even/odd planes: pT_u[p, i] = q[i, 2p+u] ---
            pTe = sb.tile([P, QROWS], b16)
            pTo = sb.tile([P, QROWS], b16)
            for u, (Gt, pT) in enumerate(((Ge, pTe), (Go, pTo))):
                for o in range(0, QROWS, REG):
                    acc = awps.tile([P, REG], fp32, tag="proj")
                    nc.tensor.matmul(acc[:], lhsT=Gt[:, 0, :],
                                     rhs=xlT[:, 0, o:o + REG], start=True, stop=False)
                    nc.tensor.matmul(acc[:], lhsT=Gt[:, 1, :],
                                     rhs=xlT[:, 1, o:o + REG], start=False, stop=True)
                    nc.scalar.copy(pT[:, o:o + REG], acc[:])

            # --- main stream: per chunk, one ap_gather then per-node dots ---
            soff16 = 0
            for k, (c0, c1, nsl) in enumerate(chunks):
                kgi_t = kgip.tile([P, maxch16], i16, tag="kgi")
                nc.sync.dma_start(kgi_t[:, :nsl // 16],
                                  kgi_d[:, soff16 // 16:(soff16 + nsl) // 16])
                kt = ktp.tile([P, maxch, 1], i32, tag="kt")
                g = nc.gpsimd.ap_gather(
                    kt[:, :nsl, :], ktT[:], kgi_t[:, :nsl // 16],
                    P, N, 1, nsl)
                add_dep_helper(lib.ins, g.ins, sync=True, reason="lib first")
                soff16 += nsl
                ktv = kt[:].bitcast(b16)                 # [P, maxch, 2]

                ncols = c1 - c0
                nreg = -(-ncols // REG)
                rw = -(-ncols // nreg)                   # region width
                for r0 in range(c0, c1, rw):
                    r1 = min(r0 + rw, c1)
                    w = r1 - r0
                    aw = awps.tile([P, REG], fp32, tag="aw")
                    nc.vector.memset(aw[:], -30.0)
                    for i in range(r0, r1):
                        ccap = int(slotcls[i])
                        o = int(sloto[i])
                        j = i - r0
                        nc.tensor.matmul(aw[0:ccap, j:j + 1],
                                         lhsT=ktv[:, o:o + ccap, 0],
                                         rhs=pTe[:, i:i + 1],
                                         start=True, stop=False)
                        nc.tensor.matmul(aw[0:ccap, j:j + 1],
                                         lhsT=ktv[:, o:o + ccap, 1],
                                         rhs=pTo[:, i:i + 1],
                                         start=False, stop=True)
                    awm = rp.tile([P, REG], fp32, tag="awm")
                    nc.vector.tensor_tensor(out=awm[:, :w], in0=aw[:, :w],
                                            in1=mB[:, r0:r1], op=Alu.add)
                    ex = rp.tile([P, REG], fp32, tag="ex")
                    nc.scalar.activation(ex[:, :w], awm[:, :w],
                                         mybir.ActivationFunctionType.Exp)
                    es = esps.tile([1, REG], fp32, tag="es")
                    nc.tensor.matmul(es[0:1, :w], lhsT=onesf[:, 0:1],
                                     rhs=ex[:, :w], start=True, stop=True)
                    rec = recp.tile([1, REG], fp32, tag="rec")
                    nc.vector.reciprocal_approx_fast(rec[0:1, :w], es[0:1, :w])
                    rb = rbps.tile([P, REG], fp32, tag="rb")
                    nc.tensor.matmul(rb[:, :w], lhsT=ones1[:],
                                     rhs=rec[0:1, :w], start=True, stop=True)
                    o_t = rp.tile([P, REG], fp32, tag="awm")
                    nc.vector.tensor_tensor(out=o_t[:, :w], in0=ex[:, :w],
                                            in1=rb[:, :w], op=Alu.mult)
                    eng = nc.sync if (r0 // rw) % 2 == 0 else nc.scalar
                    eng.dma_start(out_d[:, r0:r1], o_t[:, :w])
    nc.compile()
    return nc


def kernel(x, ei, W):
    global _compiled
    in_maps, unshard = _host_prep(x, ei, W)
    if _compiled is None:
        _compiled = _build()
    nc = _compiled
    from concourse.bass_utils import run_bass_kernel_spmd
    res = run_bass_kernel_spmd(nc, in_maps, core_ids=list(range(NCORES)))
    out = np.empty(E, np.float32)
    for c in range(NCORES):
        order, starts_l, d = unshard[c]
        o = res.results[c]["out"]
        for i in range(NLOC):
            n = order[i]
            dg = int(d[n])
            out[starts_l[n]:starts_l[n] + dg] = o[:dg, i]
    return out
